# revision 28
# baseline (speedup 1.0000x reference)
"""AttentionAggregator GNN kernel for 8x Trainium2 NeuronCores.

Sharding: rows (destinations) are split across 8 cores; each core computes the
full node-feature table (vw | a_neigh) for gathers, processes its own edges
(sorted by destination block), and emits its 12.5k output rows.
"""

import os
import numpy as np
import ml_dtypes

BF16 = ml_dtypes.bfloat16

# ---------------------------------------------------------------- config

def make_cfg(N=100000, DIN=128, DOUT=128, H=4, ncores=8):
    cfg = {}
    cfg["N"] = N
    cfg["DIN"] = DIN
    cfg["DOUT"] = DOUT
    cfg["H"] = H
    cfg["DH"] = DOUT // H
    cfg["ncores"] = ncores
    cfg["rows_per_core"] = N // ncores
    cfg["NCHUNKS"] = 4                      # int16 gather index range split
    cfg["CHUNK"] = N // 4
    assert N % 4 == 0 and cfg["CHUNK"] <= 32768
    cfg["NB"] = -(-cfg["rows_per_core"] // 128)   # dest blocks per core
    cfg["TROW"] = 256                        # table row elems (bf16) = 512B
    cfg["EPS"] = 1e-9
    return cfg


# ---------------------------------------------------------------- host prep

def _host_prep(inputs, cfg):
    N = cfg["N"]; H = cfg["H"]; DIN = cfg["DIN"]; DOUT = cfg["DOUT"]
    DH = cfg["DH"]; RPC = cfg["rows_per_core"]; NB = cfg["NB"]
    CHUNK = cfg["CHUNK"]; NCH = cfg["NCHUNKS"]; ncores = cfg["ncores"]

    vecs = np.ascontiguousarray(np.asarray(inputs["vecs"], np.float32))
    vals = np.asarray(inputs["adj_vals"], np.float32)
    rows = np.asarray(inputs["adj_rows"], np.int64)
    cols = np.asarray(inputs["adj_cols"], np.int64)
    W0 = np.asarray(inputs["W0"], np.float32)
    W1 = np.asarray(inputs["W1"], np.float32)      # [H, DIN, DH]
    att0 = np.asarray(inputs["att0"], np.float32)  # [H, DH]
    att1 = np.asarray(inputs["att1"], np.float32)

    # folded weight matrix [DIN, DOUT + DOUT + H + H]
    W1cat = np.transpose(W1, (1, 0, 2)).reshape(DIN, DOUT)
    A1 = np.einsum("hdk,hk->dh", W1, att1)
    A0 = np.einsum("hdk,hk->dh", W1, att0)
    Wcat = np.concatenate([W0, W1cat, A1, A0], axis=1)  # [DIN, 2*DOUT+2H]
    WCOLS = Wcat.shape[1]

    meta = {"WCOLS": WCOLS}
    # bias / norm-param handling (all trivial for the provided inputs)
    b0 = np.asarray(inputs["b0"], np.float32)
    b1 = np.asarray(inputs["b1"], np.float32).reshape(-1)   # [H*DH] after concat
    att_b0 = np.asarray(inputs["att_b0"], np.float32)
    att_b1 = np.asarray(inputs["att_b1"], np.float32)
    bcat = np.concatenate([b0, np.zeros(DOUT, np.float32), att_b1, att_b0])
    meta["use_bcat"] = bool(np.any(bcat != 0.0))
    b1cat = np.asarray(inputs["b1"], np.float32).reshape(H, DH)
    b1cat = b1cat.reshape(-1)  # concat heads -> [DOUT]
    meta["use_b1"] = bool(np.any(b1cat != 0.0))
    off0 = np.asarray(inputs["off0"], np.float32).reshape(-1)
    sc0 = np.asarray(inputs["sc0"], np.float32).reshape(-1)
    off1 = np.asarray(inputs["off1"], np.float32).reshape(-1)
    sc1 = np.asarray(inputs["sc1"], np.float32).reshape(-1)
    meta["use_ns0"] = bool(np.any(off0 != 0.0) or np.any(sc0 != 1.0))
    meta["use_ns1"] = bool(np.any(off1 != 0.0) or np.any(sc1 != 1.0))
    meta["neg_vals"] = bool(np.any(vals < 0.0))

    # ---- per-core edge grouping -------------------------------------
    per_core = []
    cnts = np.zeros((ncores, NB * NCH), np.int64)
    for d in range(ncores):
        lo = d * RPC; hi = lo + RPC
        m = (rows >= lo) & (rows < hi)
        er = rows[m] - lo
        ec = cols[m]
        ev = vals[m]
        # remap cols to per-core table slots: local nodes occupy slots [0,RPC)
        slot = np.where(ec >= hi, ec, np.where(ec < lo, ec + RPC, ec - lo))
        blk = er >> 7
        rl = (er & 127).astype(np.float32)
        ch = slot // CHUNK
        cidx = (slot % CHUNK).astype(np.int16)
        key = blk * NCH + ch
        order = np.argsort(key, kind="stable")
        key_s = key[order]
        cnts[d] = np.bincount(key_s, minlength=NB * NCH)
        per_core.append((rl[order], ev[order], cidx[order], key_s))

    tiles_bg = -(-cnts.max(axis=0) // 128)        # [NB*NCH] shared static sizes
    tiles_bg = tiles_bg.reshape(NB, NCH)
    Tb = tiles_bg.sum(axis=1)                      # tiles per block
    # skip blocks with zero tiles entirely (shouldn't happen at full scale)
    toff = np.concatenate([[0], np.cumsum(Tb)]).astype(np.int64)
    TT = int(toff[-1])                             # total tiles per core

    meta["tiles_bg"] = tiles_bg
    meta["Tb"] = Tb
    meta["toff"] = toff
    meta["TT"] = TT

    # ---- build padded per-core arrays -------------------------------
    core_arrays = []
    for d in range(ncores):
        rl_s, ev_s, ci_s, key_s = per_core[d]
        starts = np.concatenate([[0], np.cumsum(cnts[d])]).astype(np.int64)
        rl_pad = np.zeros(TT * 128, np.float32)
        ev_pad = np.zeros(TT * 128, np.float32)
        ci_pad = np.zeros(TT * 128, np.int16)
        pos = 0
        for b in range(NB):
            for g in range(NCH):
                k = b * NCH + g
                n = int(cnts[d][k]); cap = int(tiles_bg[b, g]) * 128
                if cap == 0:
                    continue
                sl = slice(starts[k], starts[k] + n)
                rl_pad[pos:pos + n] = rl_s[sl]
                ev_pad[pos:pos + n] = ev_s[sl]
                ci_pad[pos:pos + n] = ci_s[sl]
                pos += cap
        assert pos == TT * 128
        # layouts
        rl_t = rl_pad.reshape(TT, 128).T.copy()            # [128, TT]
        ev_t = ev_pad.reshape(TT, 128).T.copy()            # [128, TT]
        # S^T one-hot [r, e]: depends only on adj_rows (host index data)
        st = (rl_pad[None, :] == np.arange(128, dtype=np.float32)[:, None])
        st = st.astype(BF16)                               # [128, TT*128]
        idxw = np.tile(ci_pad.reshape(-1, 16).T, (8, 1)).copy()  # [128, TT*8]
        core_arrays.append({
            "rl": rl_t, "val": ev_t, "st": st, "idx16": idxw,
        })

    # ---- constants ---------------------------------------------------
    consts = {
        "wcat": Wcat.astype(BF16),
        "iota_row": np.tile(np.arange(128, dtype=BF16)[None, :], (128, 1)),
        "ident": np.eye(128, dtype=BF16),
    }
    if meta["use_bcat"]:
        consts["bcat"] = bcat[None, :].astype(BF16)
        consts["ones_row"] = np.ones((1, 128), BF16)
    if meta["use_b1"]:
        consts["b1_bc"] = np.tile(b1cat[None, :], (128, 1)).astype(np.float32)
    if meta["use_ns0"]:
        consts["sc0_bc"] = np.tile(sc0[None, :], (128, 1)).astype(np.float32)
        consts["off0_bc"] = np.tile(off0[None, :], (128, 1)).astype(np.float32)
    if meta["use_ns1"]:
        consts["sc1_bc"] = np.tile(sc1[None, :], (128, 1)).astype(np.float32)
        consts["off1_bc"] = np.tile(off1[None, :], (128, 1)).astype(np.float32)

    # ---- per-core permuted vecs -------------------------------------
    vecs_cores = []
    for d in range(ncores):
        lo = d * RPC; hi = lo + RPC
        vp = np.concatenate([vecs[lo:hi], vecs[:lo], vecs[hi:]], axis=0)
        vecs_cores.append(np.ascontiguousarray(vp))

    in_maps = []
    for d in range(ncores):
        m = {"vecs": vecs_cores[d]}
        m.update(consts)
        m.update(core_arrays[d])
        in_maps.append(m)
    return in_maps, meta


# ---------------------------------------------------------------- program

def _build_program(meta, cfg):
    import concourse.bass as bass
    import concourse.mybir as mybir
    from concourse import tile

    f32 = mybir.dt.float32
    bf16 = mybir.dt.bfloat16
    i16 = mybir.dt.int16
    AF = mybir.ActivationFunctionType
    OP = mybir.AluOpType

    N = cfg["N"]; NB = cfg["NB"]; TROW = cfg["TROW"]; CHUNK = cfg["CHUNK"]
    NCH = cfg["NCHUNKS"]; WCOLS = meta["WCOLS"]; DOUT = cfg["DOUT"]
    EPS = cfg["EPS"]
    tiles_bg = meta["tiles_bg"]; Tb = meta["Tb"]; toff = meta["toff"]
    TT = meta["TT"]
    NT = -(-N // 128)                 # node tiles
    LB = 8                            # phase-1 load batch (tiles per DMA)

    from concourse.bacc import Bacc
    nc = Bacc()
    P = 128

    vecs_d = nc.dram_tensor("vecs", [N, cfg["DIN"]], f32, kind="ExternalInput")
    wcat_d = nc.dram_tensor("wcat", [P, WCOLS], bf16, kind="ExternalInput")
    iota_row_d = nc.dram_tensor("iota_row", [P, P], bf16, kind="ExternalInput")
    ident_d = nc.dram_tensor("ident", [P, P], bf16, kind="ExternalInput")
    rl_d = nc.dram_tensor("rl", [P, TT], f32, kind="ExternalInput")
    val_d = nc.dram_tensor("val", [P, TT], f32, kind="ExternalInput")
    st_d = nc.dram_tensor("st", [P, TT * P], bf16, kind="ExternalInput")
    idx_d = nc.dram_tensor("idx16", [P, TT * 8], i16, kind="ExternalInput")
    out_d = nc.dram_tensor("out", [NB * P, DOUT], f32, kind="ExternalOutput")
    extra = {}
    if meta["use_bcat"]:
        extra["bcat"] = nc.dram_tensor("bcat", [1, WCOLS], bf16, kind="ExternalInput")
        extra["ones_row"] = nc.dram_tensor("ones_row", [1, P], bf16, kind="ExternalInput")
    for nm in ("b1_bc", "sc0_bc", "off0_bc", "sc1_bc", "off1_bc"):
        if meta.get("use_b1") and nm == "b1_bc" or \
           meta.get("use_ns0") and nm in ("sc0_bc", "off0_bc") or \
           meta.get("use_ns1") and nm in ("sc1_bc", "off1_bc"):
            extra[nm] = nc.dram_tensor(nm, [P, DOUT], f32, kind="ExternalInput")

    with tile.TileContext(nc) as tc:
        with (
            tc.tile_pool(name="const", bufs=1) as cp,
            tc.tile_pool(name="res", bufs=1) as rp,
            tc.tile_pool(name="dram", bufs=1, space="DRAM") as dp,
        ):
            wcat = cp.tile([P, WCOLS], bf16)
            nc.sync.dma_start(wcat[:], wcat_d[:])
            iota_row = cp.tile([P, P], bf16)
            nc.sync.dma_start(iota_row[:], iota_row_d[:])
            ident = cp.tile([P, P], bf16)
            nc.sync.dma_start(ident[:], ident_d[:])
            ct = {}
            for nm, dt_ in extra.items():
                if nm == "bcat":
                    shp, dty = [1, WCOLS], bf16
                elif nm == "ones_row":
                    shp, dty = [1, P], bf16
                else:
                    shp, dty = [P, DOUT], f32
                ct[nm] = cp.tile(shp, dty)
                nc.sync.dma_start(ct[nm][:], dt_[:])

            table = dp.tile([N, TROW], bf16)
            ret_self = rp.tile([P, NB * P], f32)
            as_bf = rp.tile([P, NB * 4], bf16)

            nreg_cache = {}

            def nreg(v):
                if v not in nreg_cache:
                    nreg_cache[v] = nc.gpsimd.to_reg(v)
                return nreg_cache[v]

            # ================= phase 1: node table ======================
            with (
                tc.tile_pool(name="p1", bufs=3) as p1,
                tc.tile_pool(name="p1x", bufs=6) as p1x,
                tc.tile_pool(name="p1n", bufs=3) as p1n,
                tc.tile_pool(name="p1p", bufs=3, space="PSUM") as p1p,
            ):
                nbatches = -(-NT // LB)
                for bt in range(nbatches):
                    t0 = bt * LB
                    ntl = min(LB, NT - t0)
                    rows_tot = min(N - t0 * P, ntl * P)
                    xb = p1.tile([P, LB, P], bf16, tag="xb")
                    src = vecs_d[t0 * P: t0 * P + rows_tot, :]
                    if rows_tot % P == 0:
                        nc.gpsimd.dma_start(
                            xb[:, :ntl, :], src.rearrange("(t p) d -> p t d", p=P))
                    else:
                        full = rows_tot // P
                        if full:
                            nc.gpsimd.dma_start(
                                xb[:, :full, :],
                                vecs_d[t0 * P: t0 * P + full * P, :]
                                .rearrange("(t p) d -> p t d", p=P))
                        rem = rows_tot - full * P
                        nc.gpsimd.dma_start(
                            xb[:rem, full, :],
                            vecs_d[t0 * P + full * P: t0 * P + rows_tot, :])
                    tbl = p1.tile([P, LB, 256], bf16, tag="tbl")
                    nc.gpsimd.memset(tbl[:, :ntl, 136:256], 0.0)
                    # PE-transpose tiles in groups of 4 into one PSUM bank,
                    # then one batched drain to SBUF (bf16).
                    xTs = p1x.tile([P, LB * P], bf16, tag="xTs")
                    for q0 in range(0, ntl, 4):
                        qn = min(4, ntl - q0)
                        xps = p1p.tile([P, 4 * P], bf16, tag="xps")
                        cols = 0
                        for t in range(q0, q0 + qn):
                            rows = min(P, N - (t0 + t) * P)
                            nc.tensor.transpose(
                                xps[:, (t - q0) * P:(t - q0) * P + rows],
                                xb[:rows, t, :], ident[:rows, :rows])
                            cols = (t - q0) * P + rows
                        nc.scalar.activation(xTs[:, q0 * P:q0 * P + cols],
                                             xps[:, :cols], AF.Copy)
                    for t in range(ntl):
                        nt_ = t0 + t
                        rows = min(P, N - nt_ * P)
                        xT = xTs[:, t * P:t * P + rows]
                        op = p1p.tile([P, WCOLS], f32, tag="op")
                        if meta["use_bcat"]:
                            nc.tensor.matmul(op[:rows, :], xT, wcat[:],
                                             start=True, stop=False)
                            nc.tensor.matmul(op[:rows, :],
                                             ct["ones_row"][:, :rows],
                                             ct["bcat"][:], start=False, stop=True)
                        else:
                            nc.tensor.matmul(op[:rows, :], xT, wcat[:],
                                             start=True, stop=True)
                        # table row = [vw | an | as] -> bf16
                        nc.scalar.activation(tbl[:rows, t, 0:136],
                                             op[:rows, P:P + 136], AF.Copy)
                        if nt_ < NB:
                            # ---- local: ret_self + a_self ----
                            y0 = p1n.tile([P, P], f32, tag="y0")
                            s1 = p1n.tile([P, 1], f32, tag="s1")
                            nc.scalar.activation(y0[:], op[:, 0:P], AF.Relu,
                                                 accum_out=s1[:])
                            ysq = p1n.tile([P, P], f32, tag="ysq")
                            s2 = p1n.tile([P, 1], f32, tag="s2")
                            nc.scalar.activation(ysq[:], y0[:], AF.Square,
                                                 accum_out=s2[:])
                            mean = p1n.tile([P, 1], f32, tag="mean")
                            nc.vector.tensor_scalar(mean[:], s1[:], 1.0 / P, None,
                                                    OP.mult)
                            var = p1n.tile([P, 1], f32, tag="var")
                            nc.vector.tensor_scalar(var[:], s2[:], 1.0 / P, None,
                                                    OP.mult)
                            m2 = p1n.tile([P, 1], f32, tag="m2")
                            nc.vector.tensor_scalar(m2[:], mean[:], mean[:, 0:1],
                                                    None, OP.mult)
                            nc.vector.tensor_scalar(var[:], var[:], m2[:, 0:1],
                                                    EPS, OP.subtract, OP.add)
                            sq = p1n.tile([P, 1], f32, tag="sq")
                            nc.scalar.activation(sq[:], var[:], AF.Sqrt)
                            rstd = p1n.tile([P, 1], f32, tag="rstd")
                            nc.vector.reciprocal(rstd[:], sq[:])
                            dst = ret_self[:, nt_ * P:(nt_ + 1) * P]
                            nc.vector.tensor_scalar(dst, y0[:], mean[:, 0:1],
                                                    rstd[:, 0:1], OP.subtract,
                                                    OP.mult)
                            if meta["use_ns0"]:
                                nc.vector.tensor_tensor(dst, dst, ct["sc0_bc"][:],
                                                        OP.mult)
                                nc.vector.tensor_tensor(dst, dst, ct["off0_bc"][:],
                                                        OP.add)
                            nc.vector.tensor_copy(as_bf[:, nt_ * 4:(nt_ + 1) * 4],
                                                  op[:, P + 132:P + 136])
                    # batched table write (full tiles; remainder separately)
                    full = rows_tot // P
                    if full:
                        nc.sync.dma_start(
                            table[t0 * P: t0 * P + full * P, :]
                            .rearrange("(t p) c -> p t c", p=P),
                            tbl[:, :full, :])
                    rem = rows_tot - full * P
                    if rem:
                        nc.sync.dma_start(
                            table[t0 * P + full * P: t0 * P + rows_tot, :],
                            tbl[:rem, full, :])

            tc.strict_bb_all_engine_barrier()

            # ================= phase 2: edges ==========================
            with (
                tc.tile_pool(name="p2", bufs=2) as p2,
                tc.tile_pool(name="p2g", bufs=2) as p2g,
                tc.tile_pool(name="p2s", bufs=4) as p2s,
                tc.tile_pool(name="p2n", bufs=2) as p2n,
                tc.tile_pool(name="aggp", bufs=2, space="PSUM") as aggp,
                tc.tile_pool(name="xp", bufs=4, space="PSUM") as xp,
            ):
                for b in range(NB):
                    T = int(Tb[b])
                    if T == 0:
                        continue
                    o = int(toff[b])
                    idx = p2.tile([P, T * 8], i16, tag="idx")
                    nc.sync.dma_start(idx[:], idx_d[:, o * 8:(o + T) * 8])
                    rlv = p2.tile([P, T], f32, tag="rlv")
                    nc.sync.dma_start(rlv[:], rl_d[:, o:o + T])
                    valv = p2.tile([P, T], f32, tag="valv")
                    nc.sync.dma_start(valv[:], val_d[:, o:o + T])
                    st = p2.tile([P, T * P], bf16, tag="st")
                    nc.sync.dma_start(st[:], st_d[:, o * P:(o + T) * P])
                    gt = p2g.tile([P, T, TROW], bf16, tag="gt")
                    tg0 = 0
                    for g in range(NCH):
                        Tg = int(tiles_bg[b, g])
                        if Tg == 0:
                            continue
                        nc.gpsimd.dma_gather(
                            gt[:, tg0:tg0 + Tg, :],
                            table[g * CHUNK:(g + 1) * CHUNK, :],
                            idx[:, tg0 * 8:(tg0 + Tg) * 8],
                            Tg * P, nreg(Tg * P), TROW)
                        tg0 += Tg
                    agg = aggp.tile([P, P], f32, tag="agg")
                    ablk = as_bf[:, b * 4:(b + 1) * 4]
                    for t in range(T):
                        s = p2s.tile([P, P], bf16, tag="s")
                        nc.vector.tensor_scalar(s[:], iota_row[:], rlv[:, t:t + 1],
                                                None, OP.is_equal)
                        px = xp.tile([P, 4], f32, tag="px")
                        nc.tensor.matmul(px[:], st[:, t * P:(t + 1) * P], ablk,
                                         start=True, stop=False)
                        nc.tensor.matmul(px[:], ident[:], gt[:, t, P:P + 4],
                                         start=False, stop=True)
                        alpha = p2s.tile([P, 4], f32, tag="alpha")
                        if meta["neg_vals"]:
                            nc.scalar.activation(alpha[:], px[:], AF.Relu)
                            nc.vector.tensor_scalar(alpha[:], alpha[:],
                                                    valv[:, t:t + 1], None, OP.mult)
                        else:
                            nc.scalar.activation(alpha[:], px[:], AF.Relu,
                                                 scale=valv[:, t:t + 1])
                        msg = p2s.tile([P, P], bf16, tag="msg")
                        for h in range(4):
                            nc.vector.tensor_scalar(
                                msg[:, h * 32:(h + 1) * 32],
                                gt[:, t, h * 32:(h + 1) * 32],
                                alpha[:, h:h + 1], None, OP.mult)
                        nc.tensor.matmul(agg[:], s[:], msg[:], start=(t == 0),
                                         stop=(t == T - 1))
                    # ---- drain: relu, b1, rownorm, + ret_self ----
                    y = p2n.tile([P, P], f32, tag="y")
                    s1 = p2n.tile([P, 1], f32, tag="s1")
                    if meta["use_b1"]:
                        ytmp = p2n.tile([P, P], f32, tag="ytmp")
                        nc.scalar.activation(ytmp[:], agg[:], AF.Relu)
                        nc.vector.tensor_tensor(y[:], ytmp[:], ct["b1_bc"][:],
                                                OP.add)
                        ycp = p2n.tile([P, P], f32, tag="ycp")
                        nc.scalar.activation(ycp[:], y[:], AF.Copy,
                                             accum_out=s1[:])
                    else:
                        nc.scalar.activation(y[:], agg[:], AF.Relu,
                                             accum_out=s1[:])
                    ysq = p2n.tile([P, P], f32, tag="ysq")
                    s2 = p2n.tile([P, 1], f32, tag="s2")
                    nc.scalar.activation(ysq[:], y[:], AF.Square,
                                         accum_out=s2[:])
                    mean = p2n.tile([P, 1], f32, tag="mean")
                    nc.vector.tensor_scalar(mean[:], s1[:], 1.0 / P, None, OP.mult)
                    var = p2n.tile([P, 1], f32, tag="var")
                    nc.vector.tensor_scalar(var[:], s2[:], 1.0 / P, None, OP.mult)
                    m2 = p2n.tile([P, 1], f32, tag="m2")
                    nc.vector.tensor_scalar(m2[:], mean[:], mean[:, 0:1], None,
                                            OP.mult)
                    nc.vector.tensor_scalar(var[:], var[:], m2[:, 0:1], EPS,
                                            OP.subtract, OP.add)
                    sq = p2n.tile([P, 1], f32, tag="sq")
                    nc.scalar.activation(sq[:], var[:], AF.Sqrt)
                    rstd = p2n.tile([P, 1], f32, tag="rstd")
                    nc.vector.reciprocal(rstd[:], sq[:])
                    ob = p2n.tile([P, P], f32, tag="ob")
                    nc.vector.tensor_scalar(ob[:], y[:], mean[:, 0:1],
                                            rstd[:, 0:1], OP.subtract, OP.mult)
                    if meta["use_ns1"]:
                        nc.vector.tensor_tensor(ob[:], ob[:], ct["sc1_bc"][:],
                                                OP.mult)
                        nc.vector.tensor_tensor(ob[:], ob[:], ct["off1_bc"][:],
                                                OP.add)
                    nc.vector.tensor_tensor(ob[:], ob[:],
                                            ret_self[:, b * P:(b + 1) * P], OP.add)
                    nc.sync.dma_start(out_d[b * P:(b + 1) * P, :], ob[:])

    nc.finalize()
    return nc


# ---------------------------------------------------------------- entry

LAST_EXEC_NS = None
LAST_RESULTS = None


def _ensure_axon_ntff_hook():
    """Register the NTFF-profile hook that this image's antenv lacks."""
    try:
        from antenv.axon_hooks import get_axon_ntff_profile_hook  # noqa: F401
        return
    except ImportError:
        pass
    import sys
    import types
    mod = types.ModuleType("antenv.axon_hooks")
    store = {}
    mod.set_axon_ntff_profile_hook = lambda h: store.__setitem__("h", h)
    mod.get_axon_ntff_profile_hook = lambda: store.get("h")
    sys.modules["antenv.axon_hooks"] = mod
    try:
        from trn_agent_boot.trn_boot import _ntff_profile_via_ctypes
        h = _ntff_profile_via_ctypes("/opt/axon/libaxon_pjrt.so")
        if h is not None:
            store["h"] = h
    except Exception as e:  # pragma: no cover
        print("ntff hook setup failed:", e)


def kernel(**inputs):
    global LAST_EXEC_NS, LAST_RESULTS
    cfg = make_cfg()
    in_maps, meta = _host_prep(inputs, cfg)
    nc = _build_program(meta, cfg)
    from concourse import bass_utils
    trace = os.environ.get("KERNEL_TRACE", "0") == "1"
    if trace:
        _ensure_axon_ntff_hook()
        bass_utils.upload_artifacts = lambda d: str(d)
        orig_pp = bass_utils._process_ntff_profile

        def safe_pp(*a, **k):
            try:
                return orig_pp(*a, **k)
            except Exception as e:
                print("profile processing failed:", repr(e))
                return bass_utils._NtffProfileResults()
        bass_utils._process_ntff_profile = safe_pp
    res = bass_utils.run_bass_kernel_spmd(
        nc, in_maps, core_ids=list(range(cfg["ncores"])), trace=trace,
        tmpdir=os.environ.get("KERNEL_TMPDIR"))
    LAST_EXEC_NS = res.exec_time_ns
    LAST_RESULTS = res
    RPC = cfg["rows_per_core"]
    out = np.concatenate([r["out"][:RPC] for r in res.results], axis=0)
    return out.astype(np.float32)


# revision 37
# speedup vs baseline: 1.0436x; 1.0436x over previous
"""AttentionAggregator GNN kernel for 8x Trainium2 NeuronCores.

Sharding: rows (destinations) are split across 8 cores; each core computes the
full node-feature table (vw | a_neigh) for gathers, processes its own edges
(sorted by destination block), and emits its 12.5k output rows.
"""

import os
import numpy as np
import ml_dtypes

BF16 = ml_dtypes.bfloat16

# ---------------------------------------------------------------- config

def make_cfg(N=100000, DIN=128, DOUT=128, H=4, ncores=8):
    cfg = {}
    cfg["N"] = N
    cfg["DIN"] = DIN
    cfg["DOUT"] = DOUT
    cfg["H"] = H
    cfg["DH"] = DOUT // H
    cfg["ncores"] = ncores
    cfg["rows_per_core"] = N // ncores
    cfg["NCHUNKS"] = 4                      # int16 gather index range split
    cfg["CHUNK"] = N // 4
    assert N % 4 == 0 and cfg["CHUNK"] <= 32768
    cfg["NB"] = -(-cfg["rows_per_core"] // 128)   # dest blocks per core
    cfg["TROW"] = 256                        # table row elems (bf16) = 512B
    cfg["EPS"] = 1e-9
    return cfg


# ---------------------------------------------------------------- host prep

def _host_prep(inputs, cfg):
    N = cfg["N"]; H = cfg["H"]; DIN = cfg["DIN"]; DOUT = cfg["DOUT"]
    DH = cfg["DH"]; RPC = cfg["rows_per_core"]; NB = cfg["NB"]
    CHUNK = cfg["CHUNK"]; NCH = cfg["NCHUNKS"]; ncores = cfg["ncores"]

    vecs = np.ascontiguousarray(np.asarray(inputs["vecs"], np.float32))
    vals = np.asarray(inputs["adj_vals"], np.float32)
    rows = np.asarray(inputs["adj_rows"], np.int64)
    cols = np.asarray(inputs["adj_cols"], np.int64)
    W0 = np.asarray(inputs["W0"], np.float32)
    W1 = np.asarray(inputs["W1"], np.float32)      # [H, DIN, DH]
    att0 = np.asarray(inputs["att0"], np.float32)  # [H, DH]
    att1 = np.asarray(inputs["att1"], np.float32)

    # folded weight matrix [DIN, DOUT + DOUT + H + H]
    W1cat = np.transpose(W1, (1, 0, 2)).reshape(DIN, DOUT)
    A1 = np.einsum("hdk,hk->dh", W1, att1)
    A0 = np.einsum("hdk,hk->dh", W1, att0)
    Wcat = np.concatenate([W0, W1cat, A1, A0], axis=1)  # [DIN, 2*DOUT+2H]
    WCOLS = Wcat.shape[1]

    meta = {"WCOLS": WCOLS}
    # bias / norm-param handling (all trivial for the provided inputs)
    b0 = np.asarray(inputs["b0"], np.float32)
    b1 = np.asarray(inputs["b1"], np.float32).reshape(-1)   # [H*DH] after concat
    att_b0 = np.asarray(inputs["att_b0"], np.float32)
    att_b1 = np.asarray(inputs["att_b1"], np.float32)
    bcat = np.concatenate([b0, np.zeros(DOUT, np.float32), att_b1, att_b0])
    meta["use_bcat"] = bool(np.any(bcat != 0.0))
    b1cat = np.asarray(inputs["b1"], np.float32).reshape(H, DH)
    b1cat = b1cat.reshape(-1)  # concat heads -> [DOUT]
    meta["use_b1"] = bool(np.any(b1cat != 0.0))
    off0 = np.asarray(inputs["off0"], np.float32).reshape(-1)
    sc0 = np.asarray(inputs["sc0"], np.float32).reshape(-1)
    off1 = np.asarray(inputs["off1"], np.float32).reshape(-1)
    sc1 = np.asarray(inputs["sc1"], np.float32).reshape(-1)
    meta["use_ns0"] = bool(np.any(off0 != 0.0) or np.any(sc0 != 1.0))
    meta["use_ns1"] = bool(np.any(off1 != 0.0) or np.any(sc1 != 1.0))
    meta["neg_vals"] = bool(np.any(vals < 0.0))

    # ---- per-core edge grouping -------------------------------------
    per_core = []
    cnts = np.zeros((ncores, NB * NCH), np.int64)
    for d in range(ncores):
        lo = d * RPC; hi = lo + RPC
        m = (rows >= lo) & (rows < hi)
        er = rows[m] - lo
        ec = cols[m]
        ev = vals[m]
        # remap cols to per-core table slots: local nodes occupy slots [0,RPC)
        slot = np.where(ec >= hi, ec, np.where(ec < lo, ec + RPC, ec - lo))
        blk = er >> 7
        rl = (er & 127).astype(np.float32)
        ch = slot // CHUNK
        cidx = (slot % CHUNK).astype(np.int16)
        key = blk * NCH + ch
        order = np.argsort(key, kind="stable")
        key_s = key[order]
        cnts[d] = np.bincount(key_s, minlength=NB * NCH)
        per_core.append((rl[order], ev[order], cidx[order], key_s))

    tiles_bg = -(-cnts.max(axis=0) // 128)        # [NB*NCH] shared static sizes
    tiles_bg = tiles_bg.reshape(NB, NCH)
    Tb = tiles_bg.sum(axis=1)                      # tiles per block
    # skip blocks with zero tiles entirely (shouldn't happen at full scale)
    toff = np.concatenate([[0], np.cumsum(Tb)]).astype(np.int64)
    TT = int(toff[-1])                             # total tiles per core

    meta["tiles_bg"] = tiles_bg
    meta["Tb"] = Tb
    meta["toff"] = toff
    meta["TT"] = TT

    # ---- build padded per-core arrays -------------------------------
    core_arrays = []
    for d in range(ncores):
        rl_s, ev_s, ci_s, key_s = per_core[d]
        starts = np.concatenate([[0], np.cumsum(cnts[d])]).astype(np.int64)
        rl_pad = np.zeros(TT * 128, np.float32)
        ev_pad = np.zeros(TT * 128, np.float32)
        ci_pad = np.zeros(TT * 128, np.int16)
        pos = 0
        for b in range(NB):
            for g in range(NCH):
                k = b * NCH + g
                n = int(cnts[d][k]); cap = int(tiles_bg[b, g]) * 128
                if cap == 0:
                    continue
                sl = slice(starts[k], starts[k] + n)
                rl_pad[pos:pos + n] = rl_s[sl]
                ev_pad[pos:pos + n] = ev_s[sl]
                ci_pad[pos:pos + n] = ci_s[sl]
                pos += cap
        assert pos == TT * 128
        # layouts
        rl_t = rl_pad.reshape(TT, 128).T.copy()            # [128, TT]
        ev_t = ev_pad.reshape(TT, 128).T.copy()            # [128, TT]
        # S^T one-hot [r, e]: depends only on adj_rows (host index data)
        st = (rl_pad[None, :] == np.arange(128, dtype=np.float32)[:, None])
        st = st.astype(BF16)                               # [128, TT*128]
        # S one-hot [e-part, r-free] per tile: s[p, t*128+r] = (rl[t*128+p]==r)
        s = (rl_pad.reshape(TT, 128)[:, :, None]
             == np.arange(128, dtype=np.float32)[None, None, :])
        s = np.ascontiguousarray(
            s.transpose(1, 0, 2).reshape(128, TT * 128)).astype(BF16)
        idxw = np.tile(ci_pad.reshape(-1, 16).T, (8, 1)).copy()  # [128, TT*8]
        core_arrays.append({
            "rl": rl_t, "val": ev_t, "st": st, "s": s, "idx16": idxw,
        })

    # ---- constants ---------------------------------------------------
    consts = {
        "wcat": Wcat.astype(BF16),
        "iota_row": np.tile(np.arange(128, dtype=BF16)[None, :], (128, 1)),
        "ident": np.eye(128, dtype=BF16),
    }
    if meta["use_bcat"]:
        consts["bcat"] = bcat[None, :].astype(BF16)
        consts["ones_row"] = np.ones((1, 128), BF16)
    if meta["use_b1"]:
        consts["b1_bc"] = np.tile(b1cat[None, :], (128, 1)).astype(np.float32)
    if meta["use_ns0"]:
        consts["sc0_bc"] = np.tile(sc0[None, :], (128, 1)).astype(np.float32)
        consts["off0_bc"] = np.tile(off0[None, :], (128, 1)).astype(np.float32)
    if meta["use_ns1"]:
        consts["sc1_bc"] = np.tile(sc1[None, :], (128, 1)).astype(np.float32)
        consts["off1_bc"] = np.tile(off1[None, :], (128, 1)).astype(np.float32)

    # ---- per-core permuted vecs -------------------------------------
    vecs_cores = []
    for d in range(ncores):
        lo = d * RPC; hi = lo + RPC
        vp = np.concatenate([vecs[lo:hi], vecs[:lo], vecs[hi:]], axis=0)
        vecs_cores.append(np.ascontiguousarray(vp))

    in_maps = []
    for d in range(ncores):
        m = {"vecs": vecs_cores[d]}
        m.update(consts)
        m.update(core_arrays[d])
        in_maps.append(m)
    return in_maps, meta


# ---------------------------------------------------------------- program

def _build_program(meta, cfg):
    import concourse.bass as bass
    import concourse.mybir as mybir
    from concourse import tile

    f32 = mybir.dt.float32
    bf16 = mybir.dt.bfloat16
    i16 = mybir.dt.int16
    AF = mybir.ActivationFunctionType
    OP = mybir.AluOpType

    N = cfg["N"]; NB = cfg["NB"]; TROW = cfg["TROW"]; CHUNK = cfg["CHUNK"]
    NCH = cfg["NCHUNKS"]; WCOLS = meta["WCOLS"]; DOUT = cfg["DOUT"]
    EPS = cfg["EPS"]
    tiles_bg = meta["tiles_bg"]; Tb = meta["Tb"]; toff = meta["toff"]
    TT = meta["TT"]
    NT = -(-N // 128)                 # node tiles
    LB = 8                            # phase-1 load batch (tiles per DMA)

    from concourse.bacc import Bacc
    nc = Bacc()
    P = 128

    vecs_d = nc.dram_tensor("vecs", [N, cfg["DIN"]], f32, kind="ExternalInput")
    wcat_d = nc.dram_tensor("wcat", [P, WCOLS], bf16, kind="ExternalInput")
    iota_row_d = nc.dram_tensor("iota_row", [P, P], bf16, kind="ExternalInput")
    ident_d = nc.dram_tensor("ident", [P, P], bf16, kind="ExternalInput")
    rl_d = nc.dram_tensor("rl", [P, TT], f32, kind="ExternalInput")
    val_d = nc.dram_tensor("val", [P, TT], f32, kind="ExternalInput")
    st_d = nc.dram_tensor("st", [P, TT * P], bf16, kind="ExternalInput")
    s_d = nc.dram_tensor("s", [P, TT * P], bf16, kind="ExternalInput")
    idx_d = nc.dram_tensor("idx16", [P, TT * 8], i16, kind="ExternalInput")
    out_d = nc.dram_tensor("out", [NB * P, DOUT], f32, kind="ExternalOutput")
    extra = {}
    if meta["use_bcat"]:
        extra["bcat"] = nc.dram_tensor("bcat", [1, WCOLS], bf16, kind="ExternalInput")
        extra["ones_row"] = nc.dram_tensor("ones_row", [1, P], bf16, kind="ExternalInput")
    for nm in ("b1_bc", "sc0_bc", "off0_bc", "sc1_bc", "off1_bc"):
        if meta.get("use_b1") and nm == "b1_bc" or \
           meta.get("use_ns0") and nm in ("sc0_bc", "off0_bc") or \
           meta.get("use_ns1") and nm in ("sc1_bc", "off1_bc"):
            extra[nm] = nc.dram_tensor(nm, [P, DOUT], f32, kind="ExternalInput")

    with tile.TileContext(nc) as tc:
        with (
            tc.tile_pool(name="const", bufs=1) as cp,
            tc.tile_pool(name="res", bufs=1) as rp,
            tc.tile_pool(name="dram", bufs=1, space="DRAM") as dp,
        ):
            wcat = cp.tile([P, WCOLS], bf16)
            nc.sync.dma_start(wcat[:], wcat_d[:])
            iota_row = cp.tile([P, P], bf16)
            nc.sync.dma_start(iota_row[:], iota_row_d[:])
            ident = cp.tile([P, P], bf16)
            nc.sync.dma_start(ident[:], ident_d[:])
            eps_t = cp.tile([P, 1], f32)
            nc.vector.memset(eps_t[:], EPS)
            ct = {}
            for nm, dt_ in extra.items():
                if nm == "bcat":
                    shp, dty = [1, WCOLS], bf16
                elif nm == "ones_row":
                    shp, dty = [1, P], bf16
                else:
                    shp, dty = [P, DOUT], f32
                ct[nm] = cp.tile(shp, dty)
                nc.sync.dma_start(ct[nm][:], dt_[:])

            table = dp.tile([N, TROW], bf16)
            ret_self = rp.tile([P, NB * P], f32)
            as_bf = rp.tile([P, NB * 4], bf16)

            nreg_cache = {}

            def nreg(v):
                if v not in nreg_cache:
                    nreg_cache[v] = nc.gpsimd.to_reg(v)
                return nreg_cache[v]

            # ================= phase 1: node table ======================
            with (
                tc.tile_pool(name="p1", bufs=3) as p1,
                tc.tile_pool(name="p1x", bufs=6) as p1x,
                tc.tile_pool(name="p1n", bufs=3) as p1n,
                tc.tile_pool(name="p1p", bufs=3, space="PSUM") as p1p,
            ):
                nbatches = -(-NT // LB)
                for bt in range(nbatches):
                    t0 = bt * LB
                    ntl = min(LB, NT - t0)
                    rows_tot = min(N - t0 * P, ntl * P)
                    xb = p1.tile([P, LB, P], bf16, tag="xb")
                    src = vecs_d[t0 * P: t0 * P + rows_tot, :]
                    if rows_tot % P == 0:
                        nc.gpsimd.dma_start(
                            xb[:, :ntl, :], src.rearrange("(t p) d -> p t d", p=P))
                    else:
                        full = rows_tot // P
                        if full:
                            nc.gpsimd.dma_start(
                                xb[:, :full, :],
                                vecs_d[t0 * P: t0 * P + full * P, :]
                                .rearrange("(t p) d -> p t d", p=P))
                        rem = rows_tot - full * P
                        nc.gpsimd.dma_start(
                            xb[:rem, full, :],
                            vecs_d[t0 * P + full * P: t0 * P + rows_tot, :])
                    tbl = p1.tile([P, LB, 256], bf16, tag="tbl")
                    nc.gpsimd.memset(tbl[:, :ntl, 136:256], 0.0)
                    # PE-transpose tiles in groups of 4 into one PSUM bank,
                    # then one batched drain to SBUF (bf16).
                    xTs = p1x.tile([P, LB * P], bf16, tag="xTs")
                    for q0 in range(0, ntl, 4):
                        qn = min(4, ntl - q0)
                        xps = p1p.tile([P, 4 * P], bf16, tag="xps")
                        cols = 0
                        for t in range(q0, q0 + qn):
                            rows = min(P, N - (t0 + t) * P)
                            nc.tensor.transpose(
                                xps[:, (t - q0) * P:(t - q0) * P + rows],
                                xb[:rows, t, :], ident[:rows, :rows])
                            cols = (t - q0) * P + rows
                        nc.scalar.activation(xTs[:, q0 * P:q0 * P + cols],
                                             xps[:, :cols], AF.Copy)
                    for t in range(ntl):
                        nt_ = t0 + t
                        rows = min(P, N - nt_ * P)
                        xT = xTs[:, t * P:t * P + rows]
                        op = p1p.tile([P, WCOLS], f32, tag="op")
                        if meta["use_bcat"]:
                            nc.tensor.matmul(op[:rows, :], xT, wcat[:],
                                             start=True, stop=False)
                            nc.tensor.matmul(op[:rows, :],
                                             ct["ones_row"][:, :rows],
                                             ct["bcat"][:], start=False, stop=True)
                        else:
                            nc.tensor.matmul(op[:rows, :], xT, wcat[:],
                                             start=True, stop=True)
                        # table row = [vw | an | as] -> bf16
                        nc.scalar.activation(tbl[:rows, t, 0:136],
                                             op[:rows, P:P + 136], AF.Copy)
                        if nt_ < NB:
                            # ---- local: ret_self + a_self ----
                            y0 = p1n.tile([P, P], f32, tag="y0")
                            s1 = p1n.tile([P, 1], f32, tag="s1")
                            nc.scalar.activation(y0[:], op[:, 0:P], AF.Relu,
                                                 accum_out=s1[:])
                            ysq = p1n.tile([P, P], f32, tag="ysq")
                            s2 = p1n.tile([P, 1], f32, tag="s2")
                            nc.scalar.activation(ysq[:], y0[:], AF.Square,
                                                 accum_out=s2[:])
                            mean = p1n.tile([P, 1], f32, tag="mean")
                            nc.vector.tensor_scalar(mean[:], s1[:], 1.0 / P, None,
                                                    OP.mult)
                            m2 = p1n.tile([P, 1], f32, tag="m2")
                            nc.vector.tensor_scalar(m2[:], s1[:], s1[:, 0:1],
                                                    1.0 / (P * P), OP.mult, OP.mult)
                            var = p1n.tile([P, 1], f32, tag="var")
                            nc.vector.tensor_scalar(var[:], s2[:], 1.0 / P,
                                                    m2[:, 0:1], OP.mult, OP.subtract)
                            lv = p1n.tile([P, 1], f32, tag="lv")
                            nc.scalar.activation(lv[:], var[:], AF.Ln,
                                                 bias=eps_t[:, 0:1])
                            rstd = p1n.tile([P, 1], f32, tag="rstd")
                            nc.scalar.activation(rstd[:], lv[:], AF.Exp,
                                                 scale=-0.5)
                            dst = ret_self[:, nt_ * P:(nt_ + 1) * P]
                            nc.vector.tensor_scalar(dst, y0[:], mean[:, 0:1],
                                                    rstd[:, 0:1], OP.subtract,
                                                    OP.mult)
                            if meta["use_ns0"]:
                                nc.vector.tensor_tensor(dst, dst, ct["sc0_bc"][:],
                                                        OP.mult)
                                nc.vector.tensor_tensor(dst, dst, ct["off0_bc"][:],
                                                        OP.add)
                            nc.vector.tensor_copy(as_bf[:, nt_ * 4:(nt_ + 1) * 4],
                                                  op[:, P + 132:P + 136])
                    # batched table write (full tiles; remainder separately)
                    full = rows_tot // P
                    if full:
                        nc.sync.dma_start(
                            table[t0 * P: t0 * P + full * P, :]
                            .rearrange("(t p) c -> p t c", p=P),
                            tbl[:, :full, :])
                    rem = rows_tot - full * P
                    if rem:
                        nc.sync.dma_start(
                            table[t0 * P + full * P: t0 * P + rows_tot, :],
                            tbl[:rem, full, :])

            tc.strict_bb_all_engine_barrier()

            # ================= phase 2: edges ==========================
            with (
                tc.tile_pool(name="p2", bufs=2) as p2,
                tc.tile_pool(name="p2g", bufs=2) as p2g,
                tc.tile_pool(name="p2s", bufs=4) as p2s,
                tc.tile_pool(name="p2n", bufs=2) as p2n,
                tc.tile_pool(name="aggp", bufs=2, space="PSUM") as aggp,
                tc.tile_pool(name="xp", bufs=4, space="PSUM") as xp,
            ):
                for b in range(NB):
                    T = int(Tb[b])
                    if T == 0:
                        continue
                    o = int(toff[b])
                    idx = p2.tile([P, T * 8], i16, tag="idx")
                    nc.sync.dma_start(idx[:], idx_d[:, o * 8:(o + T) * 8])
                    rlv = p2.tile([P, T], f32, tag="rlv")
                    nc.sync.dma_start(rlv[:], rl_d[:, o:o + T])
                    valv = p2.tile([P, T], f32, tag="valv")
                    nc.sync.dma_start(valv[:], val_d[:, o:o + T])
                    st = p2.tile([P, T * P], bf16, tag="st")
                    nc.sync.dma_start(st[:], st_d[:, o * P:(o + T) * P])
                    sfull = p2.tile([P, T * P], bf16, tag="sfull")
                    nc.sync.dma_start(sfull[:], s_d[:, o * P:(o + T) * P])
                    gt = p2g.tile([P, T, TROW], bf16, tag="gt")
                    tg0 = 0
                    for g in range(NCH):
                        Tg = int(tiles_bg[b, g])
                        if Tg == 0:
                            continue
                        nc.gpsimd.dma_gather(
                            gt[:, tg0:tg0 + Tg, :],
                            table[g * CHUNK:(g + 1) * CHUNK, :],
                            idx[:, tg0 * 8:(tg0 + Tg) * 8],
                            Tg * P, nreg(Tg * P), TROW)
                        tg0 += Tg
                    agg = aggp.tile([P, P], f32, tag="agg")
                    ablk = as_bf[:, b * 4:(b + 1) * 4]
                    for t in range(T):
                        px = xp.tile([P, 4], f32, tag="px")
                        nc.tensor.matmul(px[:], st[:, t * P:(t + 1) * P], ablk,
                                         start=True, stop=False)
                        nc.tensor.matmul(px[:], ident[:], gt[:, t, P:P + 4],
                                         start=False, stop=True)
                        alpha = p2s.tile([P, 4], bf16, tag="alpha")
                        if meta["neg_vals"]:
                            nc.scalar.activation(alpha[:], px[:], AF.Relu)
                            nc.vector.tensor_scalar(alpha[:], alpha[:],
                                                    valv[:, t:t + 1], None, OP.mult)
                        else:
                            nc.scalar.activation(alpha[:], px[:], AF.Relu,
                                                 scale=valv[:, t:t + 1])
                        msg = p2s.tile([P, 4, 32], bf16, tag="msg")
                        av = alpha[:, :]
                        a_bc = bass.AP(av.tensor, av.offset,
                                       [list(av.ap[0]), [1, 4], [0, 32]])
                        nc.vector.tensor_tensor(
                            msg[:], gt[:, t, 0:P].rearrange("p (h k) -> p h k", h=4),
                            a_bc, OP.mult)
                        nc.tensor.matmul(agg[:], sfull[:, t * P:(t + 1) * P],
                                         msg[:].rearrange("p h k -> p (h k)"),
                                         start=(t == 0), stop=(t == T - 1))
                    # ---- drain: relu, b1, rownorm, + ret_self ----
                    y = p2n.tile([P, P], f32, tag="y")
                    s1 = p2n.tile([P, 1], f32, tag="s1")
                    if meta["use_b1"]:
                        ytmp = p2n.tile([P, P], f32, tag="ytmp")
                        nc.scalar.activation(ytmp[:], agg[:], AF.Relu)
                        nc.vector.tensor_tensor(y[:], ytmp[:], ct["b1_bc"][:],
                                                OP.add)
                        ycp = p2n.tile([P, P], f32, tag="ycp")
                        nc.scalar.activation(ycp[:], y[:], AF.Copy,
                                             accum_out=s1[:])
                    else:
                        nc.scalar.activation(y[:], agg[:], AF.Relu,
                                             accum_out=s1[:])
                    ysq = p2n.tile([P, P], f32, tag="ysq")
                    s2 = p2n.tile([P, 1], f32, tag="s2")
                    nc.scalar.activation(ysq[:], y[:], AF.Square,
                                         accum_out=s2[:])
                    mean = p2n.tile([P, 1], f32, tag="mean")
                    nc.vector.tensor_scalar(mean[:], s1[:], 1.0 / P, None, OP.mult)
                    m2 = p2n.tile([P, 1], f32, tag="m2")
                    nc.vector.tensor_scalar(m2[:], s1[:], s1[:, 0:1],
                                            1.0 / (P * P), OP.mult, OP.mult)
                    var = p2n.tile([P, 1], f32, tag="var")
                    nc.vector.tensor_scalar(var[:], s2[:], 1.0 / P, m2[:, 0:1],
                                            OP.mult, OP.subtract)
                    lv = p2n.tile([P, 1], f32, tag="lv")
                    nc.scalar.activation(lv[:], var[:], AF.Ln,
                                         bias=eps_t[:, 0:1])
                    rstd = p2n.tile([P, 1], f32, tag="rstd")
                    nc.scalar.activation(rstd[:], lv[:], AF.Exp, scale=-0.5)
                    ob = p2n.tile([P, P], f32, tag="ob")
                    nc.vector.tensor_scalar(ob[:], y[:], mean[:, 0:1],
                                            rstd[:, 0:1], OP.subtract, OP.mult)
                    if meta["use_ns1"]:
                        nc.vector.tensor_tensor(ob[:], ob[:], ct["sc1_bc"][:],
                                                OP.mult)
                        nc.vector.tensor_tensor(ob[:], ob[:], ct["off1_bc"][:],
                                                OP.add)
                    nc.vector.tensor_tensor(ob[:], ob[:],
                                            ret_self[:, b * P:(b + 1) * P], OP.add)
                    nc.sync.dma_start(out_d[b * P:(b + 1) * P, :], ob[:])

    nc.finalize()
    return nc


# ---------------------------------------------------------------- entry

LAST_EXEC_NS = None
LAST_RESULTS = None


def _ensure_axon_ntff_hook():
    """Register the NTFF-profile hook that this image's antenv lacks."""
    try:
        from antenv.axon_hooks import get_axon_ntff_profile_hook  # noqa: F401
        return
    except ImportError:
        pass
    import sys
    import types
    mod = types.ModuleType("antenv.axon_hooks")
    store = {}
    mod.set_axon_ntff_profile_hook = lambda h: store.__setitem__("h", h)
    mod.get_axon_ntff_profile_hook = lambda: store.get("h")
    sys.modules["antenv.axon_hooks"] = mod
    try:
        from trn_agent_boot.trn_boot import _ntff_profile_via_ctypes
        h = _ntff_profile_via_ctypes("/opt/axon/libaxon_pjrt.so")
        if h is not None:
            store["h"] = h
    except Exception as e:  # pragma: no cover
        print("ntff hook setup failed:", e)


def kernel(**inputs):
    global LAST_EXEC_NS, LAST_RESULTS
    cfg = make_cfg()
    in_maps, meta = _host_prep(inputs, cfg)
    nc = _build_program(meta, cfg)
    from concourse import bass_utils
    trace = os.environ.get("KERNEL_TRACE", "0") == "1"
    if trace:
        _ensure_axon_ntff_hook()
        bass_utils.upload_artifacts = lambda d: str(d)
        orig_pp = bass_utils._process_ntff_profile

        def safe_pp(*a, **k):
            try:
                return orig_pp(*a, **k)
            except Exception as e:
                print("profile processing failed:", repr(e))
                return bass_utils._NtffProfileResults()
        bass_utils._process_ntff_profile = safe_pp
    res = bass_utils.run_bass_kernel_spmd(
        nc, in_maps, core_ids=list(range(cfg["ncores"])), trace=trace,
        tmpdir=os.environ.get("KERNEL_TMPDIR"))
    LAST_EXEC_NS = res.exec_time_ns
    LAST_RESULTS = res
    RPC = cfg["rows_per_core"]
    out = np.concatenate([r["out"][:RPC] for r in res.results], axis=0)
    return out.astype(np.float32)
